# revision 36
# baseline (speedup 1.0000x reference)
"""Trainium2 Bass kernel for nn_Block_2637109920380 (dense transformer block).

Block: pre-LN attention (16 heads, causal, scale E**-0.5) + pre-LN FFN(4E), f32 I/O.
Shapes: x [4, 2048, 1024], out [4, 2048, 1024].

Sharding across 8 NeuronCores (one SPMD program):
  - token-parallel phases (LN1, proj+residual, LN2, FFN): core c owns 1024
    contiguous tokens (global token g = 1024*c + l, row b = g // 2048).
  - head-parallel attention: core c owns heads {2c, 2c+1} over ALL tokens
    (uniform causal work per core -- no load imbalance, rank-independent IR).
  - collectives: dummy 16B AllGather at t=0 absorbs the initial all-rank
    rendezvous barrier under LN1; ONE AllGather of the full LN1 output h^T
    (2MB/rank, one skew payment); per-(batch,head) AllGathers of the
    attention output so each hides under later attention compute. The output
    projection reads its own batch+token-half columns via two
    partition_id-derived dynamic-offset DMAs.
  - QKV is computed RANK-MAJOR and interleaved with batch-major attention:
    QKV for ranks 2b,2b+1 (batch b's tokens) is emitted just before
    attention for batch b, so the tensor engine fills softmax-exp wait
    bubbles with the next batch's QKV matmuls. QKV evictions run on DVE to
    keep the scalar engine free for exp.

Numerics: QKV projections in fp8e4m3 with DoubleRow (weights scaled x64 on
the host; the q,k scales fold into the softmax exp scale, the v scale
cancels via a 64.0 ones-column in the PV matmul); attention scores/PV, the
output projection and the FFN in bf16 (fp8 FFN/proj were tried and exceed
the 2e-2 error budget or don't pay); fp32 PSUM accumulation everywhere; LN
stats, softmax denominators and residuals in fp32. Softmax without
max-subtraction (scores for this block are in [-0.7, 0.7]); the denominator
comes from the ones-column row of the PV accumulator (PSUM partition 64),
copied to SBUF, reciprocal'd with the fast approximate DVE op, broadcast
across partitions via a DRAM-roundtrip DMA, and applied as a DVE multiply.
"""

import numpy as np

import concourse.bass as bass
from concourse import bacc
import concourse.mybir as mybir
import concourse.tile as tile
from concourse.masks import make_identity

F32 = mybir.dt.float32
BF16 = mybir.dt.bfloat16
FP8 = mybir.dt.float8e4
FP8_WSCALE = 64.0
AF = mybir.ActivationFunctionType
ALU = mybir.AluOpType

DEBUG = False
FFN_FP8 = False
QKV_FP8 = True
ATT_FP8 = False


class Cfg:
    def __init__(self, T=2048):
        self.B = 4
        self.T = T
        self.E = 1024
        self.H = 16
        self.DH = 64
        self.NC = 8
        self.ALLT = self.B * self.T            # all tokens
        self.TOK = self.ALLT // self.NC        # tokens per core
        self.NTILE = self.TOK // 128           # 128-token tiles per core
        self.SC = self.T // 128                # key chunks per row
        self.KC = self.E // 128                # E chunks
        self.MC = 4 * self.E // 128            # FFN hidden chunks
        assert self.TOK % 512 == 0 and self.T % 512 == 0


def build_nc(cfg: Cfg):
    """Build the single SPMD Bass program (identical IR on all 8 cores)."""
    B, T, E, NC = cfg.B, cfg.T, cfg.E, cfg.NC
    ALLT, TOK, NTILE, SC, KC, MC = (
        cfg.ALLT, cfg.TOK, cfg.NTILE, cfg.SC, cfg.KC, cfg.MC)
    P = 128
    QW = 512                  # attention query-block width
    NQ = T // QW              # query blocks per row

    nc = bacc.Bacc(trn_type="TRN2", num_devices=NC)

    # ---- I/O ----
    QDT = FP8 if QKV_FP8 else BF16
    x8 = nc.dram_tensor("x8", [NTILE, P, E], F32, kind="ExternalInput")
    wq = nc.dram_tensor("wq", [KC, P, P], QDT, kind="ExternalInput")
    wk = nc.dram_tensor("wk", [KC, P, P], QDT, kind="ExternalInput")
    wv = nc.dram_tensor("wv", [KC, P, P], QDT, kind="ExternalInput")
    ADT = FP8 if ATT_FP8 else BF16
    wp = nc.dram_tensor("wp", [KC, P, E], ADT, kind="ExternalInput")
    # w1 pre-swizzled on host: [MC, P, KC*P], w1[m, p, k*128+c] = W1[k*128+p, m*128+c]
    FFDT = FP8 if FFN_FP8 else BF16
    w1 = nc.dram_tensor("w1", [MC, P, KC * P], FFDT, kind="ExternalInput")
    w2 = nc.dram_tensor("w2", [MC, P, E], FFDT, kind="ExternalInput")
    ln1g = nc.dram_tensor("ln1g", [P, KC], F32, kind="ExternalInput")
    ln1b = nc.dram_tensor("ln1b", [P, KC], F32, kind="ExternalInput")
    ln2g = nc.dram_tensor("ln2g", [P, KC], F32, kind="ExternalInput")
    ln2b = nc.dram_tensor("ln2b", [P, KC], F32, kind="ExternalInput")
    b1c = nc.dram_tensor("b1c", [P, MC], F32, kind="ExternalInput")
    bpv = nc.dram_tensor("bpv", [1, E], F32, kind="ExternalInput")
    b2v = nc.dram_tensor("b2v", [1, E], F32, kind="ExternalInput")
    trit = nc.dram_tensor("trit", [P, P], BF16, kind="ExternalInput")
    out8 = nc.dram_tensor("out8", [NTILE, P, E], F32, kind="ExternalOutput")
    if DEBUG:
        dbg_att = nc.dram_tensor("dbg_att", [B, P, T], BF16,
                                 kind="ExternalOutput")
        dbg_attg = nc.dram_tensor("dbg_attg", [P, NC, TOK], BF16,
                                  kind="ExternalOutput")

    # ---- internal DRAM (collectives + denominator bounce) ----
    dum_s = nc.dram_tensor("dum_s", [1, 4], F32)
    dum_a = nc.dram_tensor("dum_a", [NC, 1, 4], F32, addr_space="Shared")
    h_share = nc.dram_tensor("h_share", [E, TOK], QDT)
    h_all = nc.dram_tensor("h_all", [NC, E, TOK], QDT, addr_space="Shared")
    att_share = nc.dram_tensor("att_share", [B, 2, 2, 64, T // 2], ADT)
    att_allh = [nc.dram_tensor(f"att_all{h}", [B * 2 * NC * 64 * (T // 2)],
                               ADT, addr_space="Shared") for h in range(2)]
    den_dram = nc.dram_tensor("den_dram", [2, B, T], F32)
    groups = [list(range(NC))]

    with tile.TileContext(nc) as tc:
        with (
            tc.tile_pool(name="const", bufs=1) as const,
            tc.tile_pool(name="persist", bufs=1) as persist,
        ):
            # Absorb the initial all-rank rendezvous barrier behind LN1.
            nc.gpsimd.collective_compute(
                "AllGather", ALU.bypass, ins=[dum_s[:]], outs=[dum_a[:]],
                replica_groups=groups)

            ident = const.tile([P, P], BF16)
            make_identity(nc, ident)
            wq_sb = persist.tile([P, KC, P], QDT)
            wk_sb = persist.tile([P, KC, P], QDT)
            wv_sb = persist.tile([P, KC, P], QDT)
            wp_sb = persist.tile([P, KC, E], ADT)
            ln1g_sb = const.tile([P, KC], F32)
            ln1b_sb = const.tile([P, KC], F32)
            ln2g_sb = const.tile([P, KC], F32)
            ln2b_sb = const.tile([P, KC], F32)
            b1_sb = const.tile([P, MC], F32)
            trit_sb = const.tile([P, P], BF16)
            bp_rep = const.tile([P, E], F32)
            b2_rep = const.tile([P, E], F32)
            eps_sb = const.tile([P, 1], F32)
            zero_sb = const.tile([P, 1], F32)
            nc.vector.memset(eps_sb, 1e-5)
            nc.vector.memset(zero_sb, 0.0)
            nc.sync.dma_start(out=ln1g_sb, in_=ln1g[:, :])
            nc.sync.dma_start(out=ln1b_sb, in_=ln1b[:, :])
            nc.sync.dma_start(out=wq_sb, in_=wq.rearrange("k p c -> p k c"))
            nc.sync.dma_start(out=wk_sb, in_=wk.rearrange("k p c -> p k c"))
            nc.sync.dma_start(out=wv_sb, in_=wv.rearrange("k p c -> p k c"))

            def pbcast(ap, p=P):  # replicate a free-dim AP across p partitions
                return bass.AP(tensor=ap.tensor, offset=ap.offset,
                               ap=[[0, p]] + list(ap.ap))

            # residual stream after attention (written in the proj phase)
            x2_sb = persist.tile([P, NTILE, E], F32)

            def layernorm_to_T(src_fn, g_sb, b_sb, dstT_sb):
                """LN over E (free dim) + transpose + affine.
                src_fn(pool, tt) -> [P, E] f32 tile; writes dstT_sb
                [P, KC, TOK] (dtype of dstT_sb)."""
                with (
                    tc.tile_pool(name="ln", bufs=3) as ln,
                    tc.tile_pool(name="lnp", bufs=4, space="PSUM") as lnp,
                ):
                    for tt in range(NTILE):
                        src = src_fn(ln, tt)
                        st = ln.tile([P, 2, 6], F32, tag="st")
                        mv = ln.tile([P, 2], F32, tag="mv")
                        xv = src.rearrange("p (a b) -> p a b", a=2)
                        nc.vector.bn_stats(out=st[:, 0, :], in_=xv[:, 0, :])
                        nc.vector.bn_stats(out=st[:, 1, :], in_=xv[:, 1, :])
                        nc.vector.bn_aggr(out=mv, in_=st)
                        rstd = ln.tile([P, 1], F32, tag="rstd")
                        nc.scalar.activation(out=rstd, in_=mv[:, 1:2],
                                             func=AF.Sqrt, bias=eps_sb,
                                             scale=1.0)
                        nc.vector.reciprocal(out=rstd, in_=rstd)
                        xn = ln.tile([P, E], BF16, tag="xn")
                        nc.vector.tensor_scalar(xn, src, mv[:, 0:1], rstd,
                                                ALU.subtract, ALU.mult)
                        for kc in range(KC):
                            tp = lnp.tile([P, P], BF16, tag="tp")
                            nc.tensor.transpose(tp, xn[:, kc * P:(kc + 1) * P],
                                                ident)
                            if kc % 2 == 0:
                                nc.scalar.activation(
                                    out=dstT_sb[:, kc, tt * P:(tt + 1) * P],
                                    in_=tp, func=AF.Identity,
                                    bias=b_sb[:, kc:kc + 1],
                                    scale=g_sb[:, kc:kc + 1])
                            else:
                                nc.vector.tensor_scalar(
                                    dstT_sb[:, kc, tt * P:(tt + 1) * P], tp,
                                    g_sb[:, kc:kc + 1], b_sb[:, kc:kc + 1],
                                    ALU.mult, ALU.add)

            def ln1_src(pool, tt):
                xt = pool.tile([P, E], F32, tag="xsrc", bufs=NTILE)
                nc.sync.dma_start(out=xt, in_=x8[tt, :, :])
                return xt

            # -------- LN1 -> hT_own -> h_share -> single AllGather --------
            with tc.tile_pool(name="hT", bufs=1) as hTp:
                hT_own = hTp.tile([P, KC, TOK], QDT)
                layernorm_to_T(ln1_src, ln1g_sb, ln1b_sb, hT_own)
                nc.sync.dma_start(
                    out=h_share.rearrange("(k p) t -> p k t", p=P),
                    in_=hT_own)
                nc.gpsimd.collective_compute(
                    "AllGather", ALU.bypass, ins=[h_share[:]],
                    outs=[h_all[:]], replica_groups=groups)

            # late-needed constants + weights, after LN1's x loads are queued
            nc.sync.dma_start(out=ln2g_sb, in_=ln2g[:, :])
            nc.sync.dma_start(out=ln2b_sb, in_=ln2b[:, :])
            nc.sync.dma_start(out=b1_sb, in_=b1c[:, :])
            nc.sync.dma_start(out=trit_sb, in_=trit[:, :])

            # ---- fused rank-major QKV + batch-major causal attention ----
            scale = float(E) ** -0.5
            if QKV_FP8:
                scale /= FP8_WSCALE * FP8_WSCALE
            with tc.tile_pool(name="att_sb", bufs=1) as attsb:
                qT_sb = attsb.tile([P, ALLT], BF16)   # [2*64 d, t] global cols
                kT_sb = attsb.tile([P, ALLT], BF16)
                vaug = attsb.tile([P, ALLT // P, 2, 65], BF16)
                nc.vector.memset(vaug[:, :, :, 64:65],
                 FP8_WSCALE if QKV_FP8 else 1.0)

                with (
                    tc.tile_pool(name="hTr", bufs=4) as hTrp,
                    tc.tile_pool(name="fzp", bufs=1, space="PSUM") as fzp,
                    tc.tile_pool(name="attT", bufs=3) as attTp,
                    tc.tile_pool(name="probs", bufs=6) as prp,
                    tc.tile_pool(name="post", bufs=4) as pop,
                ):
                    DRM = mybir.MatmulPerfMode.DoubleRow

                    def qkv_mms(out_ps, lhs_sb, rhs_sb, lslice, rslice):
                        if QKV_FP8:
                            for j in range(KC // 2):
                                kk = slice(2 * j, 2 * j + 2)
                                nc.tensor.matmul(
                                    out_ps, lhs_sb[:, kk, lslice],
                                    rhs_sb[:, kk, rslice],
                                    start=(j == 0), stop=(j == KC // 2 - 1),
                                    perf_mode=DRM)
                        else:
                            for k in range(KC):
                                nc.tensor.matmul(
                                    out_ps, lhs_sb[:, k, lslice],
                                    rhs_sb[:, k, rslice],
                                    start=(k == 0), stop=(k == KC - 1))

                    def qkv_rank(rr):
                        hTr = hTrp.tile([P, KC, TOK], QDT, tag="hTr")
                        nc.sync.dma_start(
                            out=hTr,
                            in_=h_all[rr].rearrange("(k p) t -> p k t", p=P))
                        for sl in range(TOK // 512):
                            ss = slice(sl * 512, (sl + 1) * 512)
                            pq = fzp.tile([P, 512], F32, tag="qk", bufs=2)
                            qkv_mms(pq, wq_sb, hTr, slice(0, P), ss)
                            nc.vector.tensor_copy(
                                qT_sb[:, rr * TOK + sl * 512:
                                      rr * TOK + (sl + 1) * 512], pq)
                            pk2 = fzp.tile([P, 512], F32, tag="qk", bufs=2)
                            qkv_mms(pk2, wk_sb, hTr, slice(0, P), ss)
                            nc.vector.tensor_copy(
                                kT_sb[:, rr * TOK + sl * 512:
                                      rr * TOK + (sl + 1) * 512], pk2)
                        for st in range(TOK // P):
                            pv = fzp.tile([P, 512], F32, tag="qk", bufs=2)
                            for k in range(KC):
                                nc.tensor.matmul(
                                    pv[:, 0:P],
                                    hTr[:, k, st * P:(st + 1) * P],
                                    wv_sb[:, k, :],
                                    start=(k == 0), stop=(k == KC - 1))
                            nc.vector.tensor_copy(
                                vaug[:, rr * (TOK // P) + st, :, 0:64],
                                pv[:, 0:P].rearrange("p (h d) -> p h d", h=2))

                    for b in range(B):
                        qkv_rank(2 * b)
                        qkv_rank(2 * b + 1)
                        for h in range(2):
                            hr = slice(h * 64, h * 64 + 64)
                            attT = attTp.tile([64, T], ADT, tag="attT")
                            for qq in range(NQ):
                                qbase = qq * QW
                                jmax = (qbase + QW) // P
                                acc = fzp.tile([65, QW], F32, tag="acc",
                                               bufs=3)
                                for j in range(jmax):
                                    lo = max(j * P, qbase)
                                    w = qbase + QW - lo
                                    ps = fzp.tile([P, QW], F32, tag="sc",
                                                  bufs=3)
                                    nc.tensor.matmul(
                                        ps[:, :w],
                                        kT_sb[hr, b * T + j * P:
                                              b * T + (j + 1) * P],
                                        qT_sb[hr, b * T + lo:
                                              b * T + qbase + QW],
                                        start=True, stop=True)
                                    pr = prp.tile([P, QW], BF16, tag="pr")
                                    nc.scalar.activation(out=pr[:, :w],
                                                         in_=ps[:, :w],
                                                         func=AF.Exp,
                                                         bias=zero_sb,
                                                         scale=scale)
                                    if lo == j * P:  # diagonal: causal mask
                                        nc.vector.tensor_mul(pr[:, 0:P],
                                                             pr[:, 0:P],
                                                             trit_sb)
                                    nc.tensor.matmul(
                                        acc[:, lo - qbase:],
                                        vaug[:, b * SC + j, h, :],
                                        pr[:, :w],
                                        start=(j == 0), stop=(j == jmax - 1))
                                # denominator: copy row 64, approx-recip,
                                # DRAM-bounce broadcast, multiply
                                dsb = pop.tile([1, QW], F32, tag="dsb")
                                nc.vector.tensor_copy(dsb, acc[64:65, :])
                                rden = pop.tile([1, QW], F32, tag="rden")
                                nc.vector.reciprocal_approx_fast(
                                    out=rden, in_=dsb)
                                nc.sync.dma_start(
                                    out=den_dram[h, b:b + 1,
                                                 qbase:qbase + QW],
                                    in_=rden)
                                denb = pop.tile([64, QW], F32, tag="denb")
                                nc.sync.dma_start(
                                    out=denb,
                                    in_=pbcast(den_dram[h, b,
                                                        qbase:qbase + QW],
                                               p=64))
                                nc.vector.tensor_mul(
                                    attT[:, qbase:qbase + QW],
                                    acc[0:64, :], denb)
                            UH = NC * 64 * (T // 2)
                            for u in range(2):
                                nc.sync.dma_start(
                                    out=att_share[b, h, u],
                                    in_=attT[:, u * (T // 2):
                                             (u + 1) * (T // 2)])
                                blk = (b * 2 + u) * UH
                                nc.gpsimd.collective_compute(
                                    "AllGather", ALU.bypass,
                                    ins=[att_share[b, h, u]],
                                    outs=[att_allh[h][blk:blk + UH].rearrange(
                                        "(r d t) -> r d t", r=NC, d=64)],
                                    replica_groups=groups)
                            if DEBUG:
                                nc.sync.dma_start(
                                    out=dbg_att[b, hr, :], in_=attT)

            # ---------------- proj + residual -> x2 ----------------
            with (
                tc.tile_pool(name="proj", bufs=1) as prj,
                tc.tile_pool(name="projx", bufs=2) as prjx,
                tc.tile_pool(name="proj_ps", bufs=2, space="PSUM") as prjp,
            ):
                nc.sync.dma_start(out=wp_sb,
                                  in_=wp.rearrange("k p c -> p k c"))
                nc.sync.dma_start(out=bp_rep, in_=pbcast(bpv[0, :]))
                nc.sync.dma_start(out=b2_rep, in_=pbcast(b2v[0, :]))
                # own batch+token-half block of the gathered attention:
                # per-head tensor, block index pid = 2b+u, UH elements each
                UH = NC * 64 * (T // 2)
                pid = nc.sync.partition_id()
                g_reg = nc.sync.alloc_register("attoff")
                nc.sync.reg_alu(g_reg, pid, UH, ALU.mult)
                g0 = nc.sync.snap(g_reg, min_val=0,
                                  max_val=(2 * B - 1) * UH)
                attg = prj.tile([P, NC, TOK], ADT)
                for hh in range(2):
                    nc.sync.dma_start(
                        out=attg[hh * 64:(hh + 1) * 64],
                        in_=att_allh[hh][bass.ds(g0, UH)].rearrange(
                            "(r d t) -> d r t", r=NC, d=64))
                if DEBUG:
                    nc.sync.dma_start(out=dbg_attg[:, :, :], in_=attg)
                for tt in range(NTILE):
                    xt = prjx.tile([P, E], F32, tag="xt",
                                   bufs=NTILE)
                    nc.sync.dma_start(out=xt, in_=x8[tt, :, :])
                    ps = prjp.tile([P, E], F32, tag="pp")
                    for n2 in range(E // 512):
                        ns = slice(n2 * 512, (n2 + 1) * 512)
                        if ATT_FP8:
                            for ri in range(NC // 2):
                                rs = slice(2 * ri, 2 * ri + 2)
                                nc.tensor.matmul(
                                    ps[:, ns],
                                    attg[:, rs, tt * P:(tt + 1) * P],
                                    wp_sb[:, rs, ns],
                                    start=(ri == 0), stop=(ri == NC // 2 - 1),
                                    perf_mode=mybir.MatmulPerfMode.DoubleRow)
                        else:
                            for r in range(NC):
                                nc.tensor.matmul(
                                    ps[:, ns],
                                    attg[:, r, tt * P:(tt + 1) * P],
                                    wp_sb[:, r, ns],
                                    start=(r == 0), stop=(r == NC - 1))
                    nc.vector.scalar_tensor_tensor(
                        out=x2_sb[:, tt, :], in0=ps,
                        scalar=(1.0 / FP8_WSCALE) if ATT_FP8 else 0.0,
                        in1=xt,
                        op0=ALU.mult if ATT_FP8 else ALU.bypass,
                        op1=ALU.add)
                    nc.vector.tensor_add(x2_sb[:, tt, :], x2_sb[:, tt, :],
                                         bp_rep)

            # ---------------- LN2 -> h2T; FFN (token-halved) ----------------
            with (
                tc.tile_pool(name="h2T", bufs=1) as h2Tp,
                tc.tile_pool(name="w2sb2", bufs=1) as w2p2,
            ):
                h2T = h2Tp.tile([P, KC, TOK], FFDT)
                layernorm_to_T(lambda pool, tt: x2_sb[:, tt, :],
                               ln2g_sb, ln2b_sb, h2T)
                w2_sb = w2p2.tile([P, MC, E], FFDT)
                for m in range(MC):
                    nc.sync.dma_start(out=w2_sb[:, m, :], in_=w2[m, :, :])

                THT = TOK // 2  # tokens per FFN half
                with (
                    tc.tile_pool(name="ff1T", bufs=1) as ff1p,
                    tc.tile_pool(name="w1s", bufs=3) as w1s,
                    tc.tile_pool(name="ff_ps", bufs=1, space="PSUM") as ffp,
                    tc.tile_pool(name="osb", bufs=2) as osb,
                ):
                    DR = mybir.MatmulPerfMode.DoubleRow
                    for th in range(2):
                        hs = slice(th * THT, (th + 1) * THT)
                        ff1T = ff1p.tile([P, MC, THT], FFDT, tag="ff1T")
                        for m in range(MC):
                            w1m = w1s.tile([P, KC * P], FFDT, tag="w1m")
                            nc.sync.dma_start(out=w1m, in_=w1[m, :, :])
                            ps = ffp.tile([P, THT], F32, tag="f1", bufs=2)
                            if FFN_FP8:
                                for j in range(KC // 2):
                                    nc.tensor.matmul(
                                        ps,
                                        w1m[:, 2 * j * P:(2 * j + 2) * P
                                            ].rearrange("p (i c) -> p i c",
                                                        i=2),
                                        h2T[:, 2 * j:2 * j + 2, hs],
                                        start=(j == 0),
                                        stop=(j == KC // 2 - 1),
                                        perf_mode=DR)
                            else:
                                for k in range(KC):
                                    nc.tensor.matmul(
                                        ps, w1m[:, k * P:(k + 1) * P],
                                        h2T[:, k, hs],
                                        start=(k == 0), stop=(k == KC - 1))
                            nc.scalar.activation(
                                out=ff1T[:, m, :], in_=ps, func=AF.Relu,
                                bias=b1_sb[:, m:m + 1],
                                scale=(1.0 / FP8_WSCALE) if FFN_FP8 else 1.0)
                        for lt in range(THT // P):
                            tt = th * (THT // P) + lt
                            ps2 = ffp.tile([P, E], F32, tag="f2", bufs=2)
                            if FFN_FP8:
                                for m2 in range(MC // 2):
                                    for n2 in range(E // 512):
                                        ns = slice(n2 * 512, (n2 + 1) * 512)
                                        nc.tensor.matmul(
                                            ps2[:, ns],
                                            ff1T[:, 2 * m2:2 * m2 + 2,
                                                 lt * P:(lt + 1) * P],
                                            w2_sb[:, 2 * m2:2 * m2 + 2, ns],
                                            start=(m2 == 0),
                                            stop=(m2 == MC // 2 - 1),
                                            perf_mode=DR)
                            else:
                                for m in range(MC):
                                    for n2 in range(E // 512):
                                        ns = slice(n2 * 512, (n2 + 1) * 512)
                                        nc.tensor.matmul(
                                            ps2[:, ns],
                                            ff1T[:, m, lt * P:(lt + 1) * P],
                                            w2_sb[:, m, ns],
                                            start=(m == 0),
                                            stop=(m == MC - 1))
                            o = osb.tile([P, E], F32, tag="o")
                            nc.vector.scalar_tensor_tensor(
                                out=o, in0=ps2,
                                scalar=(1.0 / FP8_WSCALE) if FFN_FP8 else 0.0,
                                in1=x2_sb[:, tt, :],
                                op0=ALU.mult if FFN_FP8 else ALU.bypass,
                                op1=ALU.add)
                            nc.vector.tensor_add(o, o, b2_rep)
                            nc.sync.dma_start(out=out8[tt, :, :], in_=o)

    nc.compile()
    return nc


def host_inputs(cfg: Cfg, inputs: dict, core: int) -> dict:
    """Slice/stage full inputs for one core."""
    import ml_dtypes
    bf16 = ml_dtypes.bfloat16
    E = cfg.E
    P, KC, MC, NTILE = 128, cfg.KC, cfg.MC, cfg.NTILE

    x = np.asarray(inputs["x"], np.float32).reshape(cfg.NC, NTILE, P, E)

    def headslice(w):  # [H, E, DH] -> [KC, 128, 128] for heads 2c, 2c+1
        w = np.asarray(w, np.float32)
        pair = np.concatenate([w[2 * core], w[2 * core + 1]], axis=1)  # [E, 128]
        pair = np.ascontiguousarray(pair.reshape(KC, P, P))
        if QKV_FP8:
            return (pair * FP8_WSCALE).astype(ml_dtypes.float8_e4m3)
        return pair.astype(bf16)

    def col(v, n):  # [n*128] -> [128, n] chunk-column layout
        return np.ascontiguousarray(np.asarray(v, np.float32).reshape(n, P).T)

    w1f = np.asarray(inputs["W1"], np.float32).reshape(KC, P, MC, P)
    w1h = np.ascontiguousarray(w1f.transpose(2, 1, 0, 3).reshape(MC, P, KC * P))
    if FFN_FP8:
        fp8 = ml_dtypes.float8_e4m3
        w1h = (w1h * FP8_WSCALE).astype(fp8)
        w2h = np.ascontiguousarray(
            np.asarray(inputs["W2"], np.float32).reshape(MC, P, E)
            * FP8_WSCALE).astype(fp8)
    else:
        w1h = w1h.astype(bf16)
        w2h = np.ascontiguousarray(
            np.asarray(inputs["W2"], np.float32).reshape(MC, P, E)).astype(bf16)

    return {
        "x8": np.ascontiguousarray(x[core]),
        "wq": headslice(inputs["Wq"]),
        "wk": headslice(inputs["Wk"]),
        "wv": headslice(inputs["Wv"]),
        "wp": (np.ascontiguousarray(
            np.asarray(inputs["Wp"], np.float32).reshape(KC, P, E))
            * (FP8_WSCALE if ATT_FP8 else 1.0)).astype(
            ml_dtypes.float8_e4m3 if ATT_FP8 else bf16),
        "w1": w1h,
        "w2": w2h,
        "ln1g": col(inputs["ln1_g"], KC),
        "ln1b": col(inputs["ln1_b"], KC),
        "ln2g": col(inputs["ln2_g"], KC),
        "ln2b": col(inputs["ln2_b"], KC),
        "b1c": col(inputs["b1"], MC),
        "bpv": np.asarray(inputs["bp"], np.float32).reshape(1, E),
        "b2v": np.asarray(inputs["b2"], np.float32).reshape(1, E),
        "trit": np.triu(np.ones((P, P), np.float32)).astype(bf16),
    }


_NC_CACHE = {}


def get_nc(T=2048):
    if T not in _NC_CACHE:
        _NC_CACHE[T] = build_nc(Cfg(T))
    return _NC_CACHE[T]


def kernel(**inputs) -> np.ndarray:
    from concourse.bass_utils import run_bass_kernel_spmd

    cfg = Cfg(2048)
    nc = get_nc(cfg.T)
    core_ids = list(range(cfg.NC))
    in_maps = [host_inputs(cfg, inputs, c) for c in core_ids]
    res = run_bass_kernel_spmd(nc, in_maps, core_ids)
    outs = [res.results[c]["out8"] for c in range(cfg.NC)]
    out = np.concatenate([o.reshape(cfg.TOK, cfg.E) for o in outs], axis=0)
    return np.ascontiguousarray(
        out.reshape(cfg.B, cfg.T, cfg.E).astype(np.float32))
